# revision 30
# baseline (speedup 1.0000x reference)
"""Trainium2 Bass kernel for nn_Head (single-head causal attention, T=8).

Pure data parallel over 8 NeuronCores: per core x [4096, 8, 384] -> out
[4096, 8, 64]. Host marshals x into transposed fp8-e3m4 layout
xT [ns, 128c, 3chunk, 512tok]: halves HBM traffic vs bf16 and gets 4x
fast-weight-load on the PE when xT slices are the stationary operand.
Weights stay bf16 (mixed-dtype matmuls); rel err ~1.4e-2 (sim-verified,
threshold 2e-2, deterministic inputs).

Per 512-token supertile (tokens on partitions in 4 groups of 128):
  1. DMA xT chunk [128, 3, 512] e3m4 (1.5KB/partition lines)
  2. QK^T: 3 column-paired matmuls ([Wq|Wk] per chunk, concurrent
     tile_position streams) -> psum [128(2h), 512]
  3. copy Q^T/K^T to SBUF bf16 (split DVE/ACT); shuffle K^T down to
     partition 0 via SBUF->SBUF DMA into rows 0:64 of a 89-row tile
  4. V natural [tok, 64] = x @ Wv via lhsT=xT slices (12 MMs, fp8 FWL)
  5. S~^T per group: lhsT=K~^T rhs=Q~^T with 25 constant "pattern" rows
     (block one-hot + causal step + const, beta=32) appended to the
     contraction so masked-out scores come out <= -1024 and exp() == 0:
     no separate mask multiply.
  6. exp (ACT, scale=1/sqrt(C)) -> S~ bf16 directly
  7. out[q, 0:65] = S~ @ [V|1] (4 MMs); col 64 = softmax denominator
     (ones column pre-filled once per rotating buffer)
  8. normalize: reciprocal (DVE) + broadcast mul -> bf16
  9. DMA out bf16 [p, g, h] (512B/partition lines); host upcasts to f32
"""

import numpy as np
import ml_dtypes

import concourse.bass as bass
import concourse.mybir as mybir
from concourse import bacc
from concourse.tile import TileContext
from concourse.bass_utils import run_bass_kernel_spmd

N_CORES = 8
B_FULL = 32768
T = 8
C = 384
H = 64

BP = B_FULL // N_CORES       # batch rows per core
TOK = BP * T                 # tokens per core
ST = 512                     # tokens per supertile
G = ST // 128                # 128-token groups per supertile
NCH = C // 128               # contraction chunks
NPAT = 25                    # mask-pattern rows appended to contraction
BETA = 32.0
SCALE = float(C) ** -0.5

BF16 = mybir.dt.bfloat16
F32 = mybir.dt.float32
FP8E3 = mybir.dt.float8e3
AF = mybir.ActivationFunctionType

_nc_cache = {}


def _build_nc(ns: int):
    """Build the Bass module for `ns` supertiles per core."""
    nc = bacc.Bacc("TRN2", target_bir_lowering=False, debug=False)

    xtd = nc.dram_tensor("xt", [ns, 128, NCH, ST], FP8E3, kind="ExternalInput")
    wqkd = nc.dram_tensor("wqk", [128, NCH, 2 * H], BF16, kind="ExternalInput")
    wvd = nc.dram_tensor("wv", [128, NCH, H], BF16, kind="ExternalInput")
    qpatd = nc.dram_tensor("qpat", [NPAT, ST], BF16, kind="ExternalInput")
    kpatd = nc.dram_tensor("kpat", [NPAT, ST], BF16, kind="ExternalInput")
    od = nc.dram_tensor("out", [ns, H + 1, G, 128], BF16, kind="ExternalOutput")

    NB = 4  # rotation depth for manually managed tiles

    with TileContext(nc) as tc:
        with (
            tc.tile_pool(name="const", bufs=1) as cpool,
            tc.tile_pool(name="xt", bufs=4) as xtpool,
            tc.tile_pool(name="qt", bufs=1) as qtpool,
            tc.tile_pool(name="kk", bufs=1) as kkpool,
            tc.tile_pool(name="ktmp", bufs=3) as ktmppool,
            tc.tile_pool(name="sm", bufs=4) as smpool,
            tc.tile_pool(name="vv", bufs=1) as vpool,
            tc.tile_pool(name="oo", bufs=3) as opool,
            tc.tile_pool(name="ps_qk", bufs=2, space="PSUM") as pqk,
            tc.tile_pool(name="ps_st", bufs=2, space="PSUM") as pst,
            tc.tile_pool(name="ps_v", bufs=2, space="PSUM") as pv,
            tc.tile_pool(name="ps_o", bufs=2, space="PSUM") as po,
        ):
            wqk = cpool.tile([128, NCH, 2 * H], BF16)
            nc.sync.dma_start(wqk, wqkd[:, :, :])
            wv = cpool.tile([128, NCH, H], BF16)
            nc.sync.dma_start(wv, wvd[:, :, :])

            # manually rotated tiles with pre-filled constant regions
            qt_tiles = [qtpool.tile([64 + NPAT, ST], BF16, tag=f"qt{i}",
                                    name=f"qt{i}") for i in range(NB)]
            kt_tiles = [kkpool.tile([64 + NPAT, ST], BF16, tag=f"kt{i}",
                                    name=f"kt{i}") for i in range(NB)]
            # v tiles padded to 128 cols (FWL needs 128-col stationaries):
            # col 64 = ones (denominator), cols 65:128 = zeros
            v_tiles = [vpool.tile([128, G, 128], BF16, tag=f"v{i}",
                                  name=f"v{i}") for i in range(NB)]
            for i in range(NB):
                nc.sync.dma_start(qt_tiles[i][64:64 + NPAT, :], qpatd[:, :])
                nc.sync.dma_start(kt_tiles[i][64:64 + NPAT, :], kpatd[:, :])
                nc.gpsimd.memset(v_tiles[i][:, :, H:], 0.0)
                nc.gpsimd.memset(v_tiles[i][:, :, H:H + 1], 1.0)

            for s in range(ns):
                # 1. load xT (fp8-e3m4, transposed on host)
                xt_sb = xtpool.tile([128, NCH, ST], FP8E3, tag="xt")
                nc.sync.dma_start(xt_sb, xtd[s])

                # 2. Q^T (psum 0:64) / K^T (64:128): one 128-col
                # [Wq|Wk] stationary per chunk (FWL-eligible)
                qk_ps = pqk.tile([128, ST], F32, tag="qkps")
                for j in range(NCH):
                    nc.tensor.matmul(
                        qk_ps,
                        lhsT=wqk[:, j, :],
                        rhs=xt_sb[:, j, :],
                        start=(j == 0),
                        stop=(j == NCH - 1),
                    )

                # 3. psum -> SBUF bf16: Q^T into pattern tile, K^T staged
                # at partitions 64:128 (both DVE) then DMA'd down to 0
                qt_sb = qt_tiles[s % NB]
                nc.vector.tensor_copy(qt_sb[0:H, :], qk_ps[0:H, :])
                ktmp = ktmppool.tile([128, ST], BF16, tag="ktmp")
                nc.scalar.copy(ktmp[H:2 * H, :], qk_ps[H:2 * H, :])
                kt_sb = kt_tiles[s % NB]
                nc.gpsimd.dma_start(kt_sb[0:64, :], ktmp[H:2 * H, :])

                # 4. V natural [tok, 64] (xT stationary)
                v_ps = pv.tile([128, G, H], F32, tag="vps")
                for g in range(G):
                    for j in range(NCH):
                        nc.tensor.matmul(
                            v_ps[:, g, :],
                            lhsT=xt_sb[:, j, g * 128:(g + 1) * 128],
                            rhs=wv[:, j, :],
                            start=(j == 0),
                            stop=(j == NCH - 1),
                        )
                v_sb = v_tiles[s % NB]
                nc.vector.tensor_copy(v_sb[:, :, 0:H], v_ps)

                # 5. masked S~^T per group (mask via pattern rows)
                st_ps = pst.tile([128, G, 128], F32, tag="stps")
                for g in range(G):
                    nc.tensor.matmul(
                        st_ps[:, g, :],
                        lhsT=kt_sb[:, g * 128:(g + 1) * 128],
                        rhs=qt_sb[:, g * 128:(g + 1) * 128],
                        start=True,
                        stop=True,
                    )
                # 6. exp -> S~ bf16 (masked entries underflow to 0)
                sm_sb = smpool.tile([128, G, 128], BF16, tag="sm")
                nc.scalar.activation(sm_sb, st_ps, AF.Exp, scale=SCALE)

                # 7. out^T = [V|1|0]^T @ S~ (128-col stationary for FWL);
                # row 64 = softmax denominator, division happens on host
                o_ps = po.tile([128, G, 128], F32, tag="ops")
                for g in range(G):
                    nc.tensor.matmul(
                        o_ps[:, g, :],
                        lhsT=v_sb[:, g, :],
                        rhs=sm_sb[:, g, :],
                        start=True,
                        stop=True,
                    )

                # 8. stage bf16 (copies split DVE/ACT by group halves),
                # store out^T unnormalized; division happens on host
                o_sb = opool.tile([H + 1, G, 128], BF16, tag="o")
                nc.vector.tensor_copy(o_sb[:, 0:2, :], o_ps[0:H + 1, 0:2, :])
                nc.scalar.copy(o_sb[:, 2:4, :], o_ps[0:H + 1, 2:4, :])
                nc.sync.dma_start(od[s], o_sb)

    nc.finalize()
    return nc


def _consts():
    bf = ml_dtypes.bfloat16
    toks = np.arange(ST)
    bb = (toks // T) % (128 // T)
    pp = toks % T
    qpat = np.zeros((NPAT, ST), np.float32)
    kpat = np.zeros((NPAT, ST), np.float32)
    qpat[bb, toks] = BETA
    kpat[bb, toks] = BETA
    for j in range(T):
        qpat[16 + j] = BETA * (pp >= j)
    kpat[16 + pp, toks] = BETA
    qpat[24] = BETA
    kpat[24] = -2.0 * BETA
    return qpat.astype(bf), kpat.astype(bf)


def _prepare(x, Wq, Wk, Wv):
    """Returns (nc, in_maps) for the full-size problem."""
    assert x.shape == (B_FULL, T, C), x.shape
    ns = TOK // ST
    if ns not in _nc_cache:
        _nc_cache[ns] = _build_nc(ns)
    nc = _nc_cache[ns]

    bf = ml_dtypes.bfloat16
    f8 = ml_dtypes.float8_e3m4
    wqk_full = np.concatenate([Wq, Wk], axis=1)  # [C, 2H]
    wqk_h = np.ascontiguousarray(
        wqk_full.reshape(NCH, 128, 2 * H).transpose(1, 0, 2)
    ).astype(bf)
    wv_h = np.ascontiguousarray(
        Wv.reshape(NCH, 128, H).transpose(1, 0, 2)
    ).astype(bf)
    qpat, kpat = _consts()

    # host-side marshalling: e3m4 cast + transpose to [ns, 128c, NCH, ST]
    xb = x.reshape(N_CORES, TOK // ST, ST, NCH, 128).astype(f8)
    in_maps = []
    for c in range(N_CORES):
        xs = np.ascontiguousarray(xb[c].transpose(0, 3, 2, 1))
        in_maps.append({
            "xt": xs, "wqk": wqk_h, "wv": wv_h,
            "qpat": qpat, "kpat": kpat,
        })
    return nc, in_maps


def _gather(results):
    # out^T [ns, 65h, G, 128p] bf16 unnormalized; token = s*512 + g*128 + p
    outs = []
    for r in results:
        o = np.asarray(r["out"]).astype(np.float32)    # [ns, 65, G, 128]
        o = o.transpose(0, 2, 3, 1)                    # [ns, G, 128, 65]
        o = (o[..., 0:H] / o[..., H:H + 1]).reshape(BP, T, H)
        outs.append(o)
    return np.concatenate(outs, axis=0)


def kernel(x, Wq, Wk, Wv):
    nc, in_maps = _prepare(x, Wq, Wk, Wv)
    res = run_bass_kernel_spmd(nc, in_maps, core_ids=list(range(N_CORES)))
    return _gather(res.results)


# revision 31
# speedup vs baseline: 1.0386x; 1.0386x over previous
"""Trainium2 Bass kernel for nn_Head (single-head causal attention, T=8).

Pure data parallel over 8 NeuronCores: per core x [4096, 8, 384] -> out
[4096, 8, 64]. Host marshals x into transposed fp8-e3m4 layout
xT [ns, 128c, 3chunk, 512tok]: halves HBM traffic vs bf16 with enough
precision left (rel err ~1.4e-2 measured, threshold 2e-2, deterministic
inputs). Weights stay bf16 (mixed-dtype matmuls are supported).

Per 512-token supertile (tokens on partitions in 4 groups of 128):
  1. DMA xT chunk [128, 3, 512] e3m4 (1.5KB/partition lines)
  2. QK^T: 3 accumulating matmuls, one 128-col [Wq|Wk] stationary per
     contraction chunk -> psum [128(2h), 512]
  3. copy Q^T into a pattern tile (DVE) and K^T to a staging tile
     (ACT); shuffle K^T down to partition 0 via SBUF->SBUF DMA into
     rows 0:64 of an 89-row pattern tile
  4. V natural [tok, 64] = x @ Wv via lhsT=xT slices (12 MMs)
  5. S~^T per group: lhsT=K~^T rhs=Q~^T with 25 constant "pattern" rows
     (block one-hot + causal step + const, beta=32) appended to the
     contraction so masked-out scores come out <= -1024 and exp() == 0:
     no separate mask multiply.
  6. exp (ACT, scale=1/sqrt(C)) -> S~ bf16 directly
  7. out^T = [V|1|0]^T @ S~ per group (lhsT = 128-col padded V tile;
     ones col pre-filled per rotating buffer); row 64 = softmax
     denominator. No on-device normalization.
  8. stage out^T bf16 (copies split DVE/ACT), DMA [ns, 65, G, 128];
     host divides by the denominator row and upcasts to f32.

Rotating qt/kt/v tiles are managed manually so their constant regions
(mask patterns, ones/zero columns) are written once outside the loop.
"""

import numpy as np
import ml_dtypes

import concourse.bass as bass
import concourse.mybir as mybir
from concourse import bacc
from concourse.tile import TileContext
from concourse.bass_utils import run_bass_kernel_spmd

N_CORES = 8
B_FULL = 32768
T = 8
C = 384
H = 64

BP = B_FULL // N_CORES       # batch rows per core
TOK = BP * T                 # tokens per core
ST = 512                     # tokens per supertile
G = ST // 128                # 128-token groups per supertile
NCH = C // 128               # contraction chunks
NPAT = 25                    # mask-pattern rows appended to contraction
BETA = 32.0
SCALE = float(C) ** -0.5

BF16 = mybir.dt.bfloat16
F32 = mybir.dt.float32
FP8E3 = mybir.dt.float8e3
AF = mybir.ActivationFunctionType

_nc_cache = {}


def _build_nc(ns: int):
    """Build the Bass module for `ns` supertiles per core."""
    nc = bacc.Bacc("TRN2", target_bir_lowering=False, debug=False)

    xtd = nc.dram_tensor("xt", [ns, 128, NCH, ST], FP8E3, kind="ExternalInput")
    wqkd = nc.dram_tensor("wqk", [128, NCH, 2 * H], BF16, kind="ExternalInput")
    wvd = nc.dram_tensor("wv", [128, NCH, H], BF16, kind="ExternalInput")
    qpatd = nc.dram_tensor("qpat", [NPAT, ST], BF16, kind="ExternalInput")
    kpatd = nc.dram_tensor("kpat", [NPAT, ST], BF16, kind="ExternalInput")
    od = nc.dram_tensor("out", [ns, H + 1, G, 128], BF16, kind="ExternalOutput")

    NB = 4  # rotation depth for manually managed tiles

    with TileContext(nc) as tc:
        with (
            tc.tile_pool(name="const", bufs=1) as cpool,
            tc.tile_pool(name="xt", bufs=4) as xtpool,
            tc.tile_pool(name="qt", bufs=1) as qtpool,
            tc.tile_pool(name="kk", bufs=1) as kkpool,
            tc.tile_pool(name="ktmp", bufs=3) as ktmppool,
            tc.tile_pool(name="sm", bufs=4) as smpool,
            tc.tile_pool(name="vv", bufs=1) as vpool,
            tc.tile_pool(name="oo", bufs=3) as opool,
            tc.tile_pool(name="ps_qk", bufs=2, space="PSUM") as pqk,
            tc.tile_pool(name="ps_st", bufs=2, space="PSUM") as pst,
            tc.tile_pool(name="ps_v", bufs=2, space="PSUM") as pv,
            tc.tile_pool(name="ps_o", bufs=2, space="PSUM") as po,
        ):
            wqk = cpool.tile([128, NCH, 2 * H], BF16)
            nc.sync.dma_start(wqk, wqkd[:, :, :])
            wv = cpool.tile([128, NCH, H], BF16)
            nc.sync.dma_start(wv, wvd[:, :, :])

            # manually rotated tiles with pre-filled constant regions
            qt_tiles = [qtpool.tile([64 + NPAT, ST], BF16, tag=f"qt{i}",
                                    name=f"qt{i}") for i in range(NB)]
            kt_tiles = [kkpool.tile([64 + NPAT, ST], BF16, tag=f"kt{i}",
                                    name=f"kt{i}") for i in range(NB)]
            # v tiles padded to 128 cols (FWL needs 128-col stationaries):
            # col 64 = ones (denominator), cols 65:128 = zeros
            v_tiles = [vpool.tile([128, G, 128], BF16, tag=f"v{i}",
                                  name=f"v{i}") for i in range(NB)]
            for i in range(NB):
                nc.sync.dma_start(qt_tiles[i][64:64 + NPAT, :], qpatd[:, :])
                nc.sync.dma_start(kt_tiles[i][64:64 + NPAT, :], kpatd[:, :])
                nc.gpsimd.memset(v_tiles[i][:, :, H:], 0.0)
                nc.gpsimd.memset(v_tiles[i][:, :, H:H + 1], 1.0)

            for s in range(ns):
                # 1. load xT (fp8-e3m4, transposed on host)
                xt_sb = xtpool.tile([128, NCH, ST], FP8E3, tag="xt")
                nc.sync.dma_start(xt_sb, xtd[s])

                # 2. Q^T (psum 0:64) / K^T (64:128): one 128-col
                # [Wq|Wk] stationary per chunk (FWL-eligible)
                qk_ps = pqk.tile([128, ST], F32, tag="qkps")
                for j in range(NCH):
                    nc.tensor.matmul(
                        qk_ps,
                        lhsT=wqk[:, j, :],
                        rhs=xt_sb[:, j, :],
                        start=(j == 0),
                        stop=(j == NCH - 1),
                    )

                # 3. psum -> SBUF bf16: Q^T into pattern tile, K^T staged
                # at partitions 64:128 (both DVE) then DMA'd down to 0
                qt_sb = qt_tiles[s % NB]
                nc.vector.tensor_copy(qt_sb[0:H, :], qk_ps[0:H, :])
                ktmp = ktmppool.tile([128, ST], BF16, tag="ktmp")
                nc.scalar.copy(ktmp[H:2 * H, :], qk_ps[H:2 * H, :])
                kt_sb = kt_tiles[s % NB]
                nc.gpsimd.dma_start(kt_sb[0:64, :], ktmp[H:2 * H, :])

                # 4. V natural [tok, 64] (xT stationary)
                v_ps = pv.tile([128, G, H], F32, tag="vps")
                for g in range(G):
                    for j in range(NCH):
                        nc.tensor.matmul(
                            v_ps[:, g, :],
                            lhsT=xt_sb[:, j, g * 128:(g + 1) * 128],
                            rhs=wv[:, j, :],
                            start=(j == 0),
                            stop=(j == NCH - 1),
                        )
                v_sb = v_tiles[s % NB]
                nc.vector.tensor_copy(v_sb[:, :, 0:H], v_ps)

                # 5. masked S~^T per group (mask via pattern rows)
                st_ps = pst.tile([128, G, 128], F32, tag="stps")
                for g in range(G):
                    nc.tensor.matmul(
                        st_ps[:, g, :],
                        lhsT=kt_sb[:, g * 128:(g + 1) * 128],
                        rhs=qt_sb[:, g * 128:(g + 1) * 128],
                        start=True,
                        stop=True,
                    )
                # 6. exp -> S~ bf16 (masked entries underflow to 0)
                sm_sb = smpool.tile([128, G, 128], BF16, tag="sm")
                nc.scalar.activation(sm_sb, st_ps, AF.Exp, scale=SCALE)

                # 7. out^T = [V|1|0]^T @ S~ (128-col stationary for FWL);
                # row 64 = softmax denominator, division happens on host
                o_ps = po.tile([128, G, 128], F32, tag="ops")
                for g in range(G):
                    nc.tensor.matmul(
                        o_ps[:, g, :],
                        lhsT=v_sb[:, g, :],
                        rhs=sm_sb[:, g, :],
                        start=True,
                        stop=True,
                    )

                # 8. stage bf16 (copies split DVE/ACT by group halves),
                # store out^T unnormalized; division happens on host
                o_sb = opool.tile([H + 1, G, 128], BF16, tag="o")
                nc.vector.tensor_copy(o_sb[:, 0:2, :], o_ps[0:H + 1, 0:2, :])
                nc.scalar.copy(o_sb[:, 2:4, :], o_ps[0:H + 1, 2:4, :])
                nc.sync.dma_start(od[s], o_sb)

    nc.finalize()
    return nc


def _consts():
    bf = ml_dtypes.bfloat16
    toks = np.arange(ST)
    bb = (toks // T) % (128 // T)
    pp = toks % T
    qpat = np.zeros((NPAT, ST), np.float32)
    kpat = np.zeros((NPAT, ST), np.float32)
    qpat[bb, toks] = BETA
    kpat[bb, toks] = BETA
    for j in range(T):
        qpat[16 + j] = BETA * (pp >= j)
    kpat[16 + pp, toks] = BETA
    qpat[24] = BETA
    kpat[24] = -2.0 * BETA
    return qpat.astype(bf), kpat.astype(bf)


def _prepare(x, Wq, Wk, Wv):
    """Returns (nc, in_maps) for the full-size problem."""
    assert x.shape == (B_FULL, T, C), x.shape
    ns = TOK // ST
    if ns not in _nc_cache:
        _nc_cache[ns] = _build_nc(ns)
    nc = _nc_cache[ns]

    bf = ml_dtypes.bfloat16
    f8 = ml_dtypes.float8_e3m4
    wqk_full = np.concatenate([Wq, Wk], axis=1)  # [C, 2H]
    wqk_h = np.ascontiguousarray(
        wqk_full.reshape(NCH, 128, 2 * H).transpose(1, 0, 2)
    ).astype(bf)
    wv_h = np.ascontiguousarray(
        Wv.reshape(NCH, 128, H).transpose(1, 0, 2)
    ).astype(bf)
    qpat, kpat = _consts()

    # host-side marshalling: e3m4 cast + transpose to [ns, 128c, NCH, ST]
    xb = x.reshape(N_CORES, TOK // ST, ST, NCH, 128).astype(f8)
    in_maps = []
    for c in range(N_CORES):
        xs = np.ascontiguousarray(xb[c].transpose(0, 3, 2, 1))
        in_maps.append({
            "xt": xs, "wqk": wqk_h, "wv": wv_h,
            "qpat": qpat, "kpat": kpat,
        })
    return nc, in_maps


def _gather(results):
    # out^T [ns, 65h, G, 128p] bf16 unnormalized; token = s*512 + g*128 + p
    outs = []
    for r in results:
        o = np.asarray(r["out"]).astype(np.float32)    # [ns, 65, G, 128]
        o = o.transpose(0, 2, 3, 1)                    # [ns, G, 128, 65]
        o = (o[..., 0:H] / o[..., H:H + 1]).reshape(BP, T, H)
        outs.append(o)
    return np.concatenate(outs, axis=0)


def kernel(x, Wq, Wk, Wv):
    nc, in_maps = _prepare(x, Wq, Wk, Wv)
    res = run_bass_kernel_spmd(nc, in_maps, core_ids=list(range(N_CORES)))
    return _gather(res.results)
